# revision 34
# baseline (speedup 1.0000x reference)
"""Trainium2 Bass kernel: single-head causal attention (B=8, T=2048, E=1024, H=64).

Sharding: data-parallel over the batch dim — one batch element per NeuronCore,
8 cores, no collectives.

v9: bf16 datapath, host-side X transpose, 1024-wide paired projections.
  - The host ships X^T pre-interleaved as [128, pair, ec, t'] bf16 so each
    1024-wide column-PAIR is a contiguous 16KB-per-partition DMA line set.
    No PE transposes of X at all.
  - All matmuls run in bf16 (1 cycle/row at any moving width), accumulating
    in f32 PSUM. Projections use 1024-wide moving operands (the bf16 max)
    over column pairs — half the instruction count, LDWEIGHTS fully hidden
    under the 427ns streams.
  - PE clock (HAM p-state) is ramped by a warmup matmul burst on locally
    iota'd data (nonzero, varied — the activity monitor watches datapath
    toggling) with no DMA dependency.
  - K^T re-base to partitions 0:63 (zero-padded to 128) is a PE selector
    matmul (E64): compute engines cannot move data across partitions, and
    an SBUF-to-SBUF DMA would queue behind the xt prefetch stream.
  - Causal masking is a bf16 0/1 multiply on the exp'd scores tile (DVE 2x).

Per column-pair: projections QK^T/V^T (8+8 1024-wide matmuls), fused bias
on the PSUM->SBUF copies (DVE), E64 re-base, V^T PE-transposed to natural
[t, H+1] (ones column -> Z). Then per 512-column: scores S^T[k, q] = K_j Q^T
for causal k-chunks j (full-128 contraction against zero-padded K^T keeps
the PE HAM activity up); exp on ScalarE from PSUM with 1/sqrt(H) fused;
diagonal 128x128 blocks zero-masked on DVE; PV accumulates into O' PSUM
[65, 512] (row 64 = denominator Z) with scores running `lag` chunks ahead.
Output per core: [65, 2048] = [unnormalized O^T; Z]; the host divides by Z
and transposes during the unshard.
"""

import numpy as np
import ml_dtypes

import concourse.bass as bass
import concourse.bacc as bacc
import concourse.mybir as mybir
from concourse.tile import TileContext
from concourse.bass_utils import run_bass_kernel_spmd

T = 2048
E = 1024
H = 64
P = 128
TC = 512  # q chunk width for scores (one PSUM bank of f32)
PW = 2 * TC  # projection moving width (bf16 max 1024)
NT = T // P  # 16 t-tiles
NE = E // P  # 8 e-chunks
NTC = T // TC  # 4 t-chunks
NPR = T // PW  # 2 column pairs
NCORES = 8

F32 = mybir.dt.float32
BF16 = mybir.dt.bfloat16
AF = mybir.ActivationFunctionType
BF = ml_dtypes.bfloat16

# bf16 const block column layout (per partition)
CBH_WQK = 0  # [NE * 2H] = 1024, [e_chunk, m] with m: 0:64=Wq, 64:128=Wk
CBH_WV = CBH_WQK + NE * 2 * H  # [NE * H] = 512
CBH_MASK = CBH_WV + NE * H  # [128] causal keep mask: 1.0 (y>=p) else 0.0
CBH_IDENT = CBH_MASK + P  # [64] identity for the V^T transposes
CBH_E64 = CBH_IDENT + H  # [128] selector: E64[p, m] = (p == m + 64)
CBH_COLS = CBH_E64 + P


def pack_const_blocks(Wq, Wk, Wv, bq, bk, bv):
    cbh = np.zeros((P, CBH_COLS), dtype=BF)
    wqk = np.zeros((P, NE, 2 * H), dtype=np.float32)
    wqk[:, :, 0:H] = Wq.reshape(NE, P, H).transpose(1, 0, 2)
    wqk[:, :, H : 2 * H] = Wk.reshape(NE, P, H).transpose(1, 0, 2)
    cbh[:, CBH_WQK : CBH_WQK + NE * 2 * H] = wqk.reshape(P, NE * 2 * H).astype(BF)
    cbh[:, CBH_WV : CBH_WV + NE * H] = (
        Wv.reshape(NE, P, H).transpose(1, 0, 2).reshape(P, NE * H).astype(BF)
    )
    p_idx = np.arange(P)[:, None]
    y_idx = np.arange(P)[None, :]
    cbh[:, CBH_MASK : CBH_MASK + P] = (y_idx >= p_idx).astype(BF)
    cbh[0:H, CBH_IDENT : CBH_IDENT + H] = np.eye(H, dtype=np.float32).astype(BF)
    cbh[:, CBH_E64 : CBH_E64 + P] = (p_idx == y_idx + H).astype(BF)

    cbf = np.zeros((P, 2), dtype=np.float32)
    cbf[0:H, 0] = bq
    cbf[H : 2 * H, 0] = bk
    cbf[0:H, 1] = bv
    return cbh, cbf


def pack_xt(x):
    """[T, E] f32 -> [128, NPR*NE*PW] bf16, pair-interleaved X^T so each
    1024-wide column pair is a contiguous 16KB-per-partition DMA."""
    xt = x.T.astype(BF)  # [E, T]
    xp = xt.reshape(NE, P, NPR, PW).transpose(1, 2, 0, 3)  # [p, pr, ec, t']
    return np.ascontiguousarray(xp.reshape(P, NPR * NE * PW))


def build_kernel():
    nc = bacc.Bacc("TRN2", target_bir_lowering=False, debug=False)
    xt = nc.dram_tensor("xt", [P, NPR * NE * PW], BF16, kind="ExternalInput")
    cbh = nc.dram_tensor("cbh", [P, CBH_COLS], BF16, kind="ExternalInput")
    cbf = nc.dram_tensor("cbf", [P, 2], F32, kind="ExternalInput")
    out = nc.dram_tensor("out", [H + 1, T], F32, kind="ExternalOutput")

    with TileContext(nc) as tc:
        with (
            tc.tile_pool(name="const", bufs=1) as const,
            tc.tile_pool(name="es", bufs=6) as espool,
            tc.tile_pool(name="ps_prj", bufs=1, space="PSUM") as ps_prj,
            tc.tile_pool(name="ps_s", bufs=3, space="PSUM") as ps_s,
            tc.tile_pool(name="ps_o", bufs=1, space="PSUM") as ps_o,
        ):
            cbh_sb = const.tile([P, CBH_COLS], BF16)
            cbf_sb = const.tile([P, 2], F32)
            xt_all = const.tile([P, NPR, NE, PW], BF16)
            xt_t = xt.rearrange("p (r e t) -> p r e t", e=NE, t=PW)

            # warmup fodder: locally generated (no DMA dependency), nonzero
            # and varied so the PE activity monitor sees real toggling
            wtile = const.tile([P, TC], BF16)
            nc.gpsimd.iota(
                wtile[:],
                [[1, TC]],
                base=0,
                channel_multiplier=3,
                allow_small_or_imprecise_dtypes=True,
            )

            nc.sync.dma_start(cbh_sb[:], cbh[:])
            nc.sync.dma_start(cbf_sb[:], cbf[:])
            for e0 in range(0, NE, 2):
                nc.sync.dma_start(
                    xt_all[:, 0, e0 : e0 + 2], xt_t[:, 0, e0 : e0 + 2]
                )

            bqk_t = cbf_sb[:, 0:1]
            bv_t = cbf_sb[0:H, 1:2]
            wqk_sb = cbh_sb[:, CBH_WQK : CBH_WQK + NE * 2 * H].rearrange(
                "p (c m) -> p c m", m=2 * H
            )
            wv_sb = cbh_sb[:, CBH_WV : CBH_WV + NE * H].rearrange(
                "p (c m) -> p c m", m=H
            )
            maskk = cbh_sb[:, CBH_MASK : CBH_MASK + P]  # bf16 0/1 keep
            ident = cbh_sb[0:H, CBH_IDENT : CBH_IDENT + H]  # bf16 identity
            e64 = cbh_sb[:, CBH_E64 : CBH_E64 + P]  # row 64:128 selector

            # persistent activations
            qk_sb = const.tile([P, T], BF16)  # rows 0:64 = Q^T, 64:128 = K^T
            kt_sb = const.tile([P, T], BF16)  # K^T re-based, rows 64:128 zero
            vt_sb = const.tile([H, T], BF16)  # V^T staging
            v_sb = const.tile([P, NT, H + 1], BF16)  # V' = [V, 1] natural
            o_sb = const.tile([H + 1, T], F32)
            # ones column of V'
            nc.vector.memset(v_sb[:, :, H], 1.0)

            # HAM warmup: ramp the PE clock while DMAs stream in
            warm = ps_s.tile([P, TC], F32, tag="s")
            for _ in range(15):
                nc.tensor.matmul(
                    warm[:], wtile[0:P, 0:P], wtile[:], start=True, stop=True
                )

            scale = 1.0 / np.sqrt(np.float32(H))
            es_tiles = {}

            def emit_scores(j, c):
                k0 = j * P
                q0 = max(c * TC, k0)
                w = (c + 1) * TC - q0
                es = espool.tile([P, TC], BF16, tag="es", name=f"es{j}_{c}")
                es_tiles[(j, c)] = (es, q0, w)
                ps = ps_s.tile([P, TC], F32, tag="s", name=f"s{j}_{c}")
                nc.tensor.matmul(
                    ps[:, :w],
                    kt_sb[:, k0 : k0 + P],
                    qk_sb[:, q0 : q0 + w],
                    start=True,
                    stop=True,
                )
                nc.scalar.activation(
                    es[:, 0:w], ps[:, :w], AF.Exp, scale=float(scale)
                )
                if q0 == k0:
                    # causal mask inside the diagonal 128x128 block
                    nc.vector.tensor_tensor(
                        es[:, 0:P], es[:, 0:P], maskk, mybir.AluOpType.mult
                    )

            def emit_pv(jc, c, o_c, first, last):
                j, _ = jc
                es, q0, w = es_tiles.pop(jc)
                nc.tensor.matmul(
                    o_c[:, q0 - c * TC : q0 - c * TC + w],
                    v_sb[:, j, :],
                    es[:, 0:w],
                    start=first,
                    stop=last,
                )

            for pr in range(NPR):
                if pr == 0:
                    # stream in the second pair's x
                    nc.sync.dma_start(xt_all[:, 1], xt_t[:, 1])
                for c in range(2 * pr, 2 * pr + 2):
                    c0 = c * TC
                    h0 = (c - 2 * pr) * TC
                    # projections
                    pqk = ps_prj.tile([P, TC], F32, tag="pqk", name=f"pqk{c}")
                    pv = ps_prj.tile([H, TC], F32, tag="pv", name=f"pv{c}")
                    for ec in range(NE):
                        nc.tensor.matmul(
                            pqk[:],
                            wqk_sb[:, ec, :],
                            xt_all[:, pr, ec, h0 : h0 + TC],
                            start=(ec == 0),
                            stop=(ec == NE - 1),
                        )
                    for ec in range(NE):
                        nc.tensor.matmul(
                            pv[:],
                            wv_sb[:, ec, :],
                            xt_all[:, pr, ec, h0 : h0 + TC],
                            start=(ec == 0),
                            stop=(ec == NE - 1),
                        )
                    nc.vector.tensor_scalar_add(
                        qk_sb[:, c0 : c0 + TC], pqk[:], bqk_t
                    )
                    nc.vector.tensor_scalar_add(
                        vt_sb[:, c0 : c0 + TC], pv[:], bv_t
                    )
                    # K^T re-base via a PE selector matmul (E64 picks rows
                    # 64:128 and zero-pads the rest); reuses the pqk banks
                    pkt = ps_prj.tile([P, TC], F32, tag="pqk", name=f"pkt{c}")
                    nc.tensor.matmul(
                        pkt[:], e64, qk_sb[:, c0 : c0 + TC],
                        start=True, stop=True,
                    )
                    nc.vector.tensor_copy(kt_sb[:, c0 : c0 + TC], pkt[:])
                    # off-diagonal chunks first (cover the re-base latency);
                    # end on the narrowest diagonal chunk for a short drain
                    order = list(range(4 * c + 4))
                    o_c = ps_o.tile([H + 1, TC], F32, tag="o", name=f"o{c}")
                    lag = 3  # PV trails scores so ScalarE's exp stays hidden
                    vt_done = False
                    emitted = []
                    pv_i = 0
                    for j in order:
                        emit_scores(j, c)
                        emitted.append((j, c))
                        if not vt_done:
                            # V' transposes tucked behind the first score
                            for tt in range(4):
                                ti = 4 * c + tt
                                psv = ps_prj.tile(
                                    [P, H], BF16, tag="pv", name=f"psv{ti}"
                                )
                                nc.tensor.transpose(
                                    psv[:],
                                    vt_sb[:, ti * P : (ti + 1) * P],
                                    ident,
                                )
                                nc.vector.tensor_copy(v_sb[:, ti, 0:H], psv[:])
                            vt_done = True
                        if len(emitted) - pv_i > lag:
                            emit_pv(emitted[pv_i], c, o_c, pv_i == 0, False)
                            pv_i += 1
                    while pv_i < len(emitted):
                        j = emitted[pv_i][0]
                        emit_pv(
                            emitted[pv_i], c, o_c, pv_i == 0,
                            pv_i == len(emitted) - 1,
                        )
                        pv_i += 1
                        if c == NTC - 1 and j == 4 * c + 1:
                            # o columns [0:256) are final once the j=4c+1
                            # diagonal PV lands — ship them during the drain
                            nc.vector.tensor_copy(
                                o_sb[:, c0 : c0 + 256], o_c[:, 0:256]
                            )
                            nc.sync.dma_start(
                                out[:, c0 : c0 + 256], o_sb[:, c0 : c0 + 256]
                            )
                    if c == NTC - 1:
                        nc.vector.tensor_copy(
                            o_sb[:, c0 + 256 : c0 + TC], o_c[:, 256:TC]
                        )
                        nc.sync.dma_start(
                            out[:, c0 + 256 : c0 + TC],
                            o_sb[:, c0 + 256 : c0 + TC],
                        )
                    else:
                        nc.vector.tensor_copy(o_sb[:, c0 : c0 + TC], o_c[:])
                        nc.sync.dma_start(
                            out[:, c0 : c0 + TC], o_sb[:, c0 : c0 + TC]
                        )
    nc.compile()
    return nc


_NC_CACHE = None


def _get_nc():
    global _NC_CACHE
    if _NC_CACHE is None:
        _NC_CACHE = build_kernel()
    return _NC_CACHE


def prep_inputs(batch_x, Wk, bk, Wq, bq, Wv, bv):
    """Host-side marshaling: per-core interleaved X^T bf16 + const blocks."""
    batch_x = np.asarray(batch_x, dtype=np.float32)
    cbh, cbf = pack_const_blocks(
        np.asarray(Wq, dtype=np.float32),
        np.asarray(Wk, dtype=np.float32),
        np.asarray(Wv, dtype=np.float32),
        np.asarray(bq, dtype=np.float32),
        np.asarray(bk, dtype=np.float32),
        np.asarray(bv, dtype=np.float32),
    )
    return [
        {"xt": pack_xt(batch_x[i]), "cbh": cbh, "cbf": cbf}
        for i in range(NCORES)
    ]


def unshard(results):
    outs = []
    for i in range(NCORES):
        o = results[i]["out"]  # [65, 2048]
        outs.append((o[:H] / o[H : H + 1]).T)  # normalize + transpose
    return np.stack(outs).astype(np.float32)


def kernel(batch_x, Wk, bk, Wq, bq, Wv, bv):
    nc = _get_nc()
    in_maps = prep_inputs(batch_x, Wk, bk, Wq, bq, Wv, bv)
    res = run_bass_kernel_spmd(nc, in_maps, list(range(NCORES)))
    return unshard(res.results)


if __name__ == "__main__":
    rng = np.random.default_rng(0)
    inputs = {
        "batch_x": rng.standard_normal((NCORES, T, E), dtype=np.float32),
        "Wk": rng.standard_normal((E, H), dtype=np.float32) * 0.03,
        "bk": rng.standard_normal((H,), dtype=np.float32) * 0.03,
        "Wq": rng.standard_normal((E, H), dtype=np.float32) * 0.03,
        "bq": rng.standard_normal((H,), dtype=np.float32) * 0.03,
        "Wv": rng.standard_normal((E, H), dtype=np.float32) * 0.03,
        "bv": rng.standard_normal((H,), dtype=np.float32) * 0.03,
    }
    out = kernel(**inputs)
    print(out.shape, out.dtype)


# revision 35
# speedup vs baseline: 1.0579x; 1.0579x over previous
"""Trainium2 Bass kernel: single-head causal attention (B=8, T=2048, E=1024, H=64).

Sharding: data-parallel over the batch dim — one batch element per NeuronCore,
8 cores, no collectives.

v10: bf16 datapath, host-side X transpose, paired ScalarE exp.
  - The host ships X^T pre-interleaved as [128, c, ec, t'] bf16 so each
    512-wide q-column is ONE dma_start with an 8KB contiguous line per
    partition. No PE transposes of X at all.
  - All matmuls run in bf16 (1 cycle/row at any moving width), accumulating
    in f32 PSUM. V^T tiles are PE-transposed back to natural layout in bf16.
  - PE clock (HAM p-state) is ramped by a warmup matmul burst on locally
    iota'd data (nonzero, varied — the activity monitor watches datapath
    toggling), no DMA dependency.
  - K^T re-base to partitions 0:63 (zero-padded to 128) is a PE selector
    matmul (E64): compute engines cannot move data across partitions, and
    an SBUF-to-SBUF DMA would queue behind the xt prefetch stream.
  - Scores are emitted in PAIRS into one 2-bank PSUM tile so a single
    ScalarE exp covers both chunks — the exp stream is the pacing engine
    in the late columns, and this halves its per-call overhead.
  - Causal masking is a bf16 0/1 multiply on the exp'd scores tile (DVE 2x).

Column sweep over 512-wide q-chunks c = 0..3:
  per column: projections QK^T/V^T (8+8 matmul chains), biases fused into
  the PSUM->SBUF copies (DVE), E64 re-base, V^T transposes; then score
  chunk-pairs (off-diagonal j first — they only need this column's Q^T —
  ending on the narrow diagonal chunks for a short drain); exp per pair on
  ScalarE straight from PSUM with the 1/sqrt(H) scale fused; PV accumulates
  into O' PSUM [65, 512] (row 64 = denominator Z) trailing the scores by
  `lag` pairs. Output per core: [65, 2048] = [unnormalized O^T; Z]; host
  divides by Z and transposes during the unshard.
"""

import numpy as np
import ml_dtypes

import concourse.bass as bass
import concourse.bacc as bacc
import concourse.mybir as mybir
from concourse.tile import TileContext
from concourse.bass_utils import run_bass_kernel_spmd

T = 2048
E = 1024
H = 64
P = 128
TC = 512  # t/q chunk width (one PSUM bank of f32)
NT = T // P  # 16 t-tiles
NE = E // P  # 8 e-chunks
NTC = T // TC  # 4 t-chunks
NCORES = 8

F32 = mybir.dt.float32
BF16 = mybir.dt.bfloat16
AF = mybir.ActivationFunctionType
BF = ml_dtypes.bfloat16

# bf16 const block column layout (per partition)
CBH_WQK = 0  # [NE * 2H] = 1024, [e_chunk, m] with m: 0:64=Wq, 64:128=Wk
CBH_WV = CBH_WQK + NE * 2 * H  # [NE * H] = 512
CBH_MASK = CBH_WV + NE * H  # [128] causal keep mask: 1.0 (y>=p) else 0.0
CBH_IDENT = CBH_MASK + P  # [64] identity for the V^T transposes
CBH_E64 = CBH_IDENT + H  # [128] selector: E64[p, m] = (p == m + 64)
CBH_COLS = CBH_E64 + P


def pack_const_blocks(Wq, Wk, Wv, bq, bk, bv):
    cbh = np.zeros((P, CBH_COLS), dtype=BF)
    wqk = np.zeros((P, NE, 2 * H), dtype=np.float32)
    wqk[:, :, 0:H] = Wq.reshape(NE, P, H).transpose(1, 0, 2)
    wqk[:, :, H : 2 * H] = Wk.reshape(NE, P, H).transpose(1, 0, 2)
    cbh[:, CBH_WQK : CBH_WQK + NE * 2 * H] = wqk.reshape(P, NE * 2 * H).astype(BF)
    cbh[:, CBH_WV : CBH_WV + NE * H] = (
        Wv.reshape(NE, P, H).transpose(1, 0, 2).reshape(P, NE * H).astype(BF)
    )
    p_idx = np.arange(P)[:, None]
    y_idx = np.arange(P)[None, :]
    cbh[:, CBH_MASK : CBH_MASK + P] = (y_idx >= p_idx).astype(BF)
    cbh[0:H, CBH_IDENT : CBH_IDENT + H] = np.eye(H, dtype=np.float32).astype(BF)
    cbh[:, CBH_E64 : CBH_E64 + P] = (p_idx == y_idx + H).astype(BF)

    cbf = np.zeros((P, 2), dtype=np.float32)
    cbf[0:H, 0] = bq
    cbf[H : 2 * H, 0] = bk
    cbf[0:H, 1] = bv
    return cbh, cbf


def pack_xt(x):
    """[T, E] f32 -> [128, NTC*NE*TC] bf16, column-interleaved X^T so each
    512-wide column is one contiguous 8KB-per-partition DMA."""
    xt = x.T.astype(BF)  # [E, T]
    xp = xt.reshape(NE, P, NTC, TC).transpose(1, 2, 0, 3)  # [p, c, ec, t']
    return np.ascontiguousarray(xp.reshape(P, NTC * NE * TC))


def build_kernel():
    nc = bacc.Bacc("TRN2", target_bir_lowering=False, debug=False)
    xt = nc.dram_tensor("xt", [P, NTC * NE * TC], BF16, kind="ExternalInput")
    cbh = nc.dram_tensor("cbh", [P, CBH_COLS], BF16, kind="ExternalInput")
    cbf = nc.dram_tensor("cbf", [P, 2], F32, kind="ExternalInput")
    out = nc.dram_tensor("out", [H + 1, T], F32, kind="ExternalOutput")

    with TileContext(nc) as tc:
        with (
            tc.tile_pool(name="const", bufs=1) as const,
            tc.tile_pool(name="es", bufs=4) as espool,
            tc.tile_pool(name="ps_prj", bufs=1, space="PSUM") as ps_prj,
            tc.tile_pool(name="ps_s", bufs=2, space="PSUM") as ps_s,
            tc.tile_pool(name="ps_o", bufs=1, space="PSUM") as ps_o,
        ):
            cbh_sb = const.tile([P, CBH_COLS], BF16)
            cbf_sb = const.tile([P, 2], F32)
            xt_all = const.tile([P, NTC, NE, TC], BF16)
            xt_t = xt.rearrange("p (c e t) -> p c e t", e=NE, t=TC)

            # warmup fodder: locally generated (no DMA dependency), nonzero
            # and varied so the PE activity monitor sees real toggling
            wtile = const.tile([P, TC], BF16)
            nc.gpsimd.iota(
                wtile[:],
                [[1, TC]],
                base=0,
                channel_multiplier=3,
                allow_small_or_imprecise_dtypes=True,
            )

            nc.sync.dma_start(cbh_sb[:], cbh[:])
            nc.sync.dma_start(cbf_sb[:], cbf[:])
            nc.sync.dma_start(xt_all[:, 0, 0:4], xt_t[:, 0, 0:4])
            nc.sync.dma_start(xt_all[:, 0, 4:NE], xt_t[:, 0, 4:NE])

            bqk_t = cbf_sb[:, 0:1]
            bv_t = cbf_sb[0:H, 1:2]
            wqk_sb = cbh_sb[:, CBH_WQK : CBH_WQK + NE * 2 * H].rearrange(
                "p (c m) -> p c m", m=2 * H
            )
            wv_sb = cbh_sb[:, CBH_WV : CBH_WV + NE * H].rearrange(
                "p (c m) -> p c m", m=H
            )
            maskk = cbh_sb[:, CBH_MASK : CBH_MASK + P]  # bf16 0/1 keep
            ident = cbh_sb[0:H, CBH_IDENT : CBH_IDENT + H]  # bf16 identity
            e64 = cbh_sb[:, CBH_E64 : CBH_E64 + P]  # row 64:128 selector

            # persistent activations
            qk_sb = const.tile([P, T], BF16)  # rows 0:64 = Q^T, 64:128 = K^T
            kt_sb = const.tile([P, T], BF16)  # K^T re-based, rows 64:128 zero
            vt_sb = const.tile([H, T], BF16)  # V^T staging
            v_sb = const.tile([P, NT, H + 1], BF16)  # V' = [V, 1] natural
            o_sb = const.tile([H + 1, T], F32)
            # ones column of V'
            nc.vector.memset(v_sb[:, :, H], 1.0)

            # HAM warmup: ramp the PE clock while DMAs stream in
            warm = ps_s.tile([P, 2 * TC], F32, tag="s")
            for _ in range(15):
                nc.tensor.matmul(
                    warm[:, 0:TC], wtile[0:P, 0:P], wtile[:],
                    start=True, stop=True,
                )

            scale = 1.0 / np.sqrt(np.float32(H))
            es_tiles = {}

            def chunk_geom(j, c):
                k0 = j * P
                q0 = max(c * TC, k0)
                return k0, q0, (c + 1) * TC - q0

            def emit_score_pair(ja, jb, c):
                """Two score chunks into one 2-bank PSUM tile, one exp."""
                ps2 = ps_s.tile([P, 2 * TC], F32, tag="s", name=f"s{ja}_{c}")
                es2 = espool.tile([P, 2 * TC], BF16, tag="es", name=f"e{ja}_{c}")
                base = 0
                for j in (ja, jb):
                    if j is None:
                        continue
                    k0, q0, w = chunk_geom(j, c)
                    es_tiles[(j, c)] = (es2, base, q0, w)
                    nc.tensor.matmul(
                        ps2[:, base : base + w],
                        kt_sb[:, k0 : k0 + P],
                        qk_sb[:, q0 : q0 + w],
                        start=True,
                        stop=True,
                    )
                    base += w
                nc.scalar.activation(
                    es2[:, 0:base], ps2[:, 0:base], AF.Exp, scale=float(scale)
                )
                for j in (ja, jb):
                    if j is None:
                        continue
                    es2, b, q0, w = es_tiles[(j, c)]
                    if q0 == j * P:
                        # causal mask inside the diagonal 128x128 block
                        nc.vector.tensor_tensor(
                            es2[:, b : b + P],
                            es2[:, b : b + P],
                            maskk,
                            mybir.AluOpType.mult,
                        )

            def emit_pv(jc, c, o_c, first, last):
                j, _ = jc
                es2, b, q0, w = es_tiles.pop(jc)
                nc.tensor.matmul(
                    o_c[:, q0 - c * TC : q0 - c * TC + w],
                    v_sb[:, j, :],
                    es2[:, b : b + w],
                    start=first,
                    stop=last,
                )

            for c in range(NTC):
                c0 = c * TC
                # stream in the NEXT column's x
                if c < NTC - 1:
                    nc.sync.dma_start(xt_all[:, c + 1], xt_t[:, c + 1])
                # projections
                pqk = ps_prj.tile([P, TC], F32, tag="pqk", name=f"pqk{c}")
                pv = ps_prj.tile([H, TC], F32, tag="pv", name=f"pv{c}")
                for ec in range(NE):
                    nc.tensor.matmul(
                        pqk[:],
                        wqk_sb[:, ec, :],
                        xt_all[:, c, ec, :],
                        start=(ec == 0),
                        stop=(ec == NE - 1),
                    )
                for ec in range(NE):
                    nc.tensor.matmul(
                        pv[:],
                        wv_sb[:, ec, :],
                        xt_all[:, c, ec, :],
                        start=(ec == 0),
                        stop=(ec == NE - 1),
                    )
                nc.vector.tensor_scalar_add(qk_sb[:, c0 : c0 + TC], pqk[:], bqk_t)
                nc.vector.tensor_scalar_add(vt_sb[:, c0 : c0 + TC], pv[:], bv_t)
                # K^T re-base via a PE selector matmul (E64 picks rows 64:128
                # and zero-pads the rest); reuses the pqk PSUM bank
                pkt = ps_prj.tile([P, TC], F32, tag="pqk", name=f"pkt{c}")
                nc.tensor.matmul(
                    pkt[:], e64, qk_sb[:, c0 : c0 + TC], start=True, stop=True
                )
                nc.vector.tensor_copy(kt_sb[:, c0 : c0 + TC], pkt[:])

                # chunk pairs: off-diagonal first (they only need this
                # column's Q^T), ending on the narrow diagonal chunks
                order = list(range(4 * c + 4))
                pairs = [
                    (order[i], order[i + 1] if i + 1 < len(order) else None)
                    for i in range(0, len(order), 2)
                ]
                o_c = ps_o.tile([H + 1, TC], F32, tag="o", name=f"o{c}")
                lag = 2  # PV trails scores by `lag` pairs
                vt_done = False
                emitted = []
                pv_i = 0

                def drain_one(last_allowed):
                    nonlocal pv_i
                    j = emitted[pv_i][0]
                    emit_pv(
                        emitted[pv_i], c, o_c, pv_i == 0,
                        last_allowed and pv_i == len(emitted) - 1,
                    )
                    pv_i += 1
                    return j

                for pi, (ja, jb) in enumerate(pairs):
                    emit_score_pair(ja, jb, c)
                    emitted.append((ja, c))
                    if jb is not None:
                        emitted.append((jb, c))
                    if not vt_done:
                        # V' transposes tucked behind the first score pair
                        for tt in range(4):
                            ti = 4 * c + tt
                            psv = ps_prj.tile(
                                [P, H], BF16, tag="pv", name=f"psv{ti}"
                            )
                            nc.tensor.transpose(
                                psv[:], vt_sb[:, ti * P : (ti + 1) * P], ident
                            )
                            nc.vector.tensor_copy(v_sb[:, ti, 0:H], psv[:])
                        vt_done = True
                    while len(emitted) - pv_i > 2 * lag:
                        drain_one(False)
                while pv_i < len(emitted):
                    j = drain_one(True)
                    if c == NTC - 1 and j == 4 * c + 1:
                        # o columns [0:256) are final once the j=4c+1 diagonal
                        # PV lands — ship them while the drain finishes
                        nc.vector.tensor_copy(
                            o_sb[:, c0 : c0 + 256], o_c[:, 0:256]
                        )
                        nc.sync.dma_start(
                            out[:, c0 : c0 + 256], o_sb[:, c0 : c0 + 256]
                        )
                if c == NTC - 1:
                    nc.vector.tensor_copy(
                        o_sb[:, c0 + 256 : c0 + TC], o_c[:, 256:TC]
                    )
                    nc.sync.dma_start(
                        out[:, c0 + 256 : c0 + TC], o_sb[:, c0 + 256 : c0 + TC]
                    )
                else:
                    nc.vector.tensor_copy(o_sb[:, c0 : c0 + TC], o_c[:])
                    nc.sync.dma_start(out[:, c0 : c0 + TC], o_sb[:, c0 : c0 + TC])
    nc.compile()
    return nc


_NC_CACHE = None


def _get_nc():
    global _NC_CACHE
    if _NC_CACHE is None:
        _NC_CACHE = build_kernel()
    return _NC_CACHE


def prep_inputs(batch_x, Wk, bk, Wq, bq, Wv, bv):
    """Host-side marshaling: per-core interleaved X^T bf16 + const blocks."""
    batch_x = np.asarray(batch_x, dtype=np.float32)
    cbh, cbf = pack_const_blocks(
        np.asarray(Wq, dtype=np.float32),
        np.asarray(Wk, dtype=np.float32),
        np.asarray(Wv, dtype=np.float32),
        np.asarray(bq, dtype=np.float32),
        np.asarray(bk, dtype=np.float32),
        np.asarray(bv, dtype=np.float32),
    )
    return [
        {"xt": pack_xt(batch_x[i]), "cbh": cbh, "cbf": cbf}
        for i in range(NCORES)
    ]


def unshard(results):
    outs = []
    for i in range(NCORES):
        o = results[i]["out"]  # [65, 2048]
        outs.append((o[:H] / o[H : H + 1]).T)  # normalize + transpose
    return np.stack(outs).astype(np.float32)


def kernel(batch_x, Wk, bk, Wq, bq, Wv, bv):
    nc = _get_nc()
    in_maps = prep_inputs(batch_x, Wk, bk, Wq, bq, Wv, bv)
    res = run_bass_kernel_spmd(nc, in_maps, list(range(NCORES)))
    return unshard(res.results)


if __name__ == "__main__":
    rng = np.random.default_rng(0)
    inputs = {
        "batch_x": rng.standard_normal((NCORES, T, E), dtype=np.float32),
        "Wk": rng.standard_normal((E, H), dtype=np.float32) * 0.03,
        "bk": rng.standard_normal((H,), dtype=np.float32) * 0.03,
        "Wq": rng.standard_normal((E, H), dtype=np.float32) * 0.03,
        "bq": rng.standard_normal((H,), dtype=np.float32) * 0.03,
        "Wv": rng.standard_normal((E, H), dtype=np.float32) * 0.03,
        "bv": rng.standard_normal((H,), dtype=np.float32) * 0.03,
    }
    out = kernel(**inputs)
    print(out.shape, out.dtype)


# revision 38
# speedup vs baseline: 1.0595x; 1.0016x over previous
"""Trainium2 Bass kernel: single-head causal attention (B=8, T=2048, E=1024, H=64).

Sharding: data-parallel over the batch dim — one batch element per NeuronCore,
8 cores, no collectives.

v10: bf16 datapath, host-side X transpose, paired ScalarE exp.
  - The host ships X^T pre-interleaved as [128, c, ec, t'] bf16 so each
    512-wide q-column is ONE dma_start with an 8KB contiguous line per
    partition. No PE transposes of X at all.
  - All matmuls run in bf16 (1 cycle/row at any moving width), accumulating
    in f32 PSUM. V^T tiles are PE-transposed back to natural layout in bf16.
  - PE clock (HAM p-state) is ramped by a warmup matmul burst on locally
    iota'd data (nonzero, varied — the activity monitor watches datapath
    toggling), no DMA dependency.
  - K^T re-base to partitions 0:63 (zero-padded to 128) is a PE selector
    matmul (E64): compute engines cannot move data across partitions, and
    an SBUF-to-SBUF DMA would queue behind the xt prefetch stream.
  - Scores are emitted in PAIRS into one 2-bank PSUM tile so a single
    ScalarE exp covers both chunks — the exp stream is the pacing engine
    in the late columns, and this halves its per-call overhead.
  - Causal masking is a bf16 0/1 multiply on the exp'd scores tile (DVE 2x).

Column sweep over 512-wide q-chunks c = 0..3:
  per column: projections QK^T/V^T (8+8 matmul chains), biases fused into
  the PSUM->SBUF copies (DVE), E64 re-base, V^T transposes; then score
  chunk-pairs (off-diagonal j first — they only need this column's Q^T —
  ending on the narrow diagonal chunks for a short drain); exp per pair on
  ScalarE straight from PSUM with the 1/sqrt(H) scale fused; PV accumulates
  into O' PSUM [65, 512] (row 64 = denominator Z) trailing the scores by
  `lag` pairs. Output per core: [65, 2048] = [unnormalized O^T; Z]; host
  divides by Z and transposes during the unshard.
"""

import numpy as np
import ml_dtypes

import concourse.bass as bass
import concourse.bacc as bacc
import concourse.mybir as mybir
from concourse.tile import TileContext
from concourse.bass_utils import run_bass_kernel_spmd

T = 2048
E = 1024
H = 64
P = 128
TC = 512  # t/q chunk width (one PSUM bank of f32)
NT = T // P  # 16 t-tiles
NE = E // P  # 8 e-chunks
NTC = T // TC  # 4 t-chunks
NCORES = 8

F32 = mybir.dt.float32
BF16 = mybir.dt.bfloat16
AF = mybir.ActivationFunctionType
BF = ml_dtypes.bfloat16

# bf16 const block column layout (per partition)
CBH_WQK = 0  # [NE * 2H] = 1024, [e_chunk, m] with m: 0:64=Wq, 64:128=Wk
CBH_WV = CBH_WQK + NE * 2 * H  # [NE * H] = 512
CBH_MASK = CBH_WV + NE * H  # [128] causal keep mask: 1.0 (y>=p) else 0.0
CBH_IDENT = CBH_MASK + P  # [64] identity for the V^T transposes
CBH_E64 = CBH_IDENT + H  # [128] selector: E64[p, m] = (p == m + 64)
CBH_COLS = CBH_E64 + P


def pack_const_blocks(Wq, Wk, Wv, bq, bk, bv):
    cbh = np.zeros((P, CBH_COLS), dtype=BF)
    wqk = np.zeros((P, NE, 2 * H), dtype=np.float32)
    wqk[:, :, 0:H] = Wq.reshape(NE, P, H).transpose(1, 0, 2)
    wqk[:, :, H : 2 * H] = Wk.reshape(NE, P, H).transpose(1, 0, 2)
    cbh[:, CBH_WQK : CBH_WQK + NE * 2 * H] = wqk.reshape(P, NE * 2 * H).astype(BF)
    cbh[:, CBH_WV : CBH_WV + NE * H] = (
        Wv.reshape(NE, P, H).transpose(1, 0, 2).reshape(P, NE * H).astype(BF)
    )
    p_idx = np.arange(P)[:, None]
    y_idx = np.arange(P)[None, :]
    cbh[:, CBH_MASK : CBH_MASK + P] = (y_idx >= p_idx).astype(BF)
    cbh[0:H, CBH_IDENT : CBH_IDENT + H] = np.eye(H, dtype=np.float32).astype(BF)
    cbh[:, CBH_E64 : CBH_E64 + P] = (p_idx == y_idx + H).astype(BF)

    cbf = np.zeros((P, 2), dtype=np.float32)
    cbf[0:H, 0] = bq
    cbf[H : 2 * H, 0] = bk
    cbf[0:H, 1] = bv
    return cbh, cbf


def pack_xt(x):
    """[T, E] f32 -> [128, NTC*NE*TC] bf16, column-interleaved X^T so each
    512-wide column is one contiguous 8KB-per-partition DMA."""
    xt = x.T.astype(BF)  # [E, T]
    xp = xt.reshape(NE, P, NTC, TC).transpose(1, 2, 0, 3)  # [p, c, ec, t']
    return np.ascontiguousarray(xp.reshape(P, NTC * NE * TC))


def build_kernel():
    nc = bacc.Bacc("TRN2", target_bir_lowering=False, debug=False)
    xt = nc.dram_tensor("xt", [P, NTC * NE * TC], BF16, kind="ExternalInput")
    cbh = nc.dram_tensor("cbh", [P, CBH_COLS], BF16, kind="ExternalInput")
    cbf = nc.dram_tensor("cbf", [P, 2], F32, kind="ExternalInput")
    out = nc.dram_tensor("out", [H + 1, T], F32, kind="ExternalOutput")

    with TileContext(nc) as tc:
        with (
            tc.tile_pool(name="const", bufs=1) as const,
            tc.tile_pool(name="es", bufs=4) as espool,
            tc.tile_pool(name="ps_prj", bufs=1, space="PSUM") as ps_prj,
            tc.tile_pool(name="ps_s", bufs=2, space="PSUM") as ps_s,
            tc.tile_pool(name="ps_o", bufs=2, space="PSUM") as ps_o,
        ):
            cbh_sb = const.tile([P, CBH_COLS], BF16)
            cbf_sb = const.tile([P, 2], F32)
            xt_all = const.tile([P, NTC, NE, TC], BF16)
            xt_t = xt.rearrange("p (c e t) -> p c e t", e=NE, t=TC)

            # warmup fodder: locally generated (no DMA dependency), nonzero
            # and varied so the PE activity monitor sees real toggling
            wtile = const.tile([P, TC], BF16)
            nc.gpsimd.iota(
                wtile[:],
                [[1, TC]],
                base=0,
                channel_multiplier=3,
                allow_small_or_imprecise_dtypes=True,
            )

            nc.sync.dma_start(cbh_sb[:], cbh[:])
            nc.sync.dma_start(cbf_sb[:], cbf[:])
            nc.sync.dma_start(xt_all[:, 0, 0:4], xt_t[:, 0, 0:4])
            nc.sync.dma_start(xt_all[:, 0, 4:NE], xt_t[:, 0, 4:NE])

            bqk_t = cbf_sb[:, 0:1]
            bv_t = cbf_sb[0:H, 1:2]
            wqk_sb = cbh_sb[:, CBH_WQK : CBH_WQK + NE * 2 * H].rearrange(
                "p (c m) -> p c m", m=2 * H
            )
            wv_sb = cbh_sb[:, CBH_WV : CBH_WV + NE * H].rearrange(
                "p (c m) -> p c m", m=H
            )
            maskk = cbh_sb[:, CBH_MASK : CBH_MASK + P]  # bf16 0/1 keep
            ident = cbh_sb[0:H, CBH_IDENT : CBH_IDENT + H]  # bf16 identity
            e64 = cbh_sb[:, CBH_E64 : CBH_E64 + P]  # row 64:128 selector

            # persistent activations
            qk_sb = const.tile([P, T], BF16)  # rows 0:64 = Q^T, 64:128 = K^T
            kt_sb = const.tile([P, T], BF16)  # K^T re-based, rows 64:128 zero
            vt_sb = const.tile([H, T], BF16)  # V^T staging
            v_sb = const.tile([P, NT, H + 1], BF16)  # V' = [V, 1] natural
            o_sb = const.tile([H + 1, T], F32)
            # ones column of V'
            nc.vector.memset(v_sb[:, :, H], 1.0)

            # HAM warmup: ramp the PE clock while DMAs stream in
            warm = ps_s.tile([P, 2 * TC], F32, tag="s")
            for _ in range(15):
                nc.tensor.matmul(
                    warm[:, 0:TC], wtile[0:P, 0:P], wtile[:],
                    start=True, stop=True,
                )

            scale = 1.0 / np.sqrt(np.float32(H))
            es_tiles = {}

            def chunk_geom(j, c):
                k0 = j * P
                q0 = max(c * TC, k0)
                return k0, q0, (c + 1) * TC - q0

            def emit_score_pair(ja, jb, c):
                """Two score chunks into one 2-bank PSUM tile, one exp."""
                ps2 = ps_s.tile([P, 2 * TC], F32, tag="s", name=f"s{ja}_{c}")
                es2 = espool.tile([P, 2 * TC], BF16, tag="es", name=f"e{ja}_{c}")
                base = 0
                for j in (ja, jb):
                    if j is None:
                        continue
                    k0, q0, w = chunk_geom(j, c)
                    es_tiles[(j, c)] = (es2, base, q0, w)
                    nc.tensor.matmul(
                        ps2[:, base : base + w],
                        kt_sb[:, k0 : k0 + P],
                        qk_sb[:, q0 : q0 + w],
                        start=True,
                        stop=True,
                    )
                    base += w
                nc.scalar.activation(
                    es2[:, 0:base], ps2[:, 0:base], AF.Exp, scale=float(scale)
                )
                for j in (ja, jb):
                    if j is None:
                        continue
                    es2, b, q0, w = es_tiles[(j, c)]
                    if q0 == j * P:
                        # causal mask inside the diagonal 128x128 block
                        nc.vector.tensor_tensor(
                            es2[:, b : b + P],
                            es2[:, b : b + P],
                            maskk,
                            mybir.AluOpType.mult,
                        )

            def emit_pv(jc, c, o_c, first, last):
                j, _ = jc
                es2, b, q0, w = es_tiles.pop(jc)
                nc.tensor.matmul(
                    o_c[:, q0 - c * TC : q0 - c * TC + w],
                    v_sb[:, j, :],
                    es2[:, b : b + w],
                    start=first,
                    stop=last,
                )

            for c in range(NTC):
                c0 = c * TC
                # stream in the NEXT column's x
                if c < NTC - 1:
                    nc.sync.dma_start(xt_all[:, c + 1], xt_t[:, c + 1])
                # QK projection (the V chain is emitted after the first score
                # pair so the exp stream — the pacing engine — starts sooner)
                pqk = ps_prj.tile([P, TC], F32, tag="pqk", name=f"pqk{c}")
                for ec in range(NE):
                    nc.tensor.matmul(
                        pqk[:],
                        wqk_sb[:, ec, :],
                        xt_all[:, c, ec, :],
                        start=(ec == 0),
                        stop=(ec == NE - 1),
                    )
                nc.vector.tensor_scalar_add(qk_sb[:, c0 : c0 + TC], pqk[:], bqk_t)
                # K^T re-base via a PE selector matmul (E64 picks rows 64:128
                # and zero-pads the rest); reuses the pqk PSUM bank
                pkt = ps_prj.tile([P, TC], F32, tag="pqk", name=f"pkt{c}")
                nc.tensor.matmul(
                    pkt[:], e64, qk_sb[:, c0 : c0 + TC], start=True, stop=True
                )
                nc.vector.tensor_copy(kt_sb[:, c0 : c0 + TC], pkt[:])

                # chunk pairs: off-diagonal first (they only need this
                # column's Q^T), ending on the narrow diagonal chunks
                order = list(range(4 * c + 4))
                pairs = [
                    (order[i], order[i + 1] if i + 1 < len(order) else None)
                    for i in range(0, len(order), 2)
                ]
                o_c = ps_o.tile([H + 1, TC], F32, tag="o", name=f"o{c}")
                lag = 2  # PV trails scores by `lag` pairs
                vt_done = False
                emitted = []
                pv_i = 0

                def drain_one(last_allowed):
                    nonlocal pv_i
                    j = emitted[pv_i][0]
                    emit_pv(
                        emitted[pv_i], c, o_c, pv_i == 0,
                        last_allowed and pv_i == len(emitted) - 1,
                    )
                    pv_i += 1
                    return j

                for pi, (ja, jb) in enumerate(pairs):
                    emit_score_pair(ja, jb, c)
                    emitted.append((ja, c))
                    if jb is not None:
                        emitted.append((jb, c))
                    if not vt_done:
                        # V projection + V' transposes tucked behind the
                        # first score pair
                        pv = ps_prj.tile([H, TC], F32, tag="pv", name=f"pv{c}")
                        for ec in range(NE):
                            nc.tensor.matmul(
                                pv[:],
                                wv_sb[:, ec, :],
                                xt_all[:, c, ec, :],
                                start=(ec == 0),
                                stop=(ec == NE - 1),
                            )
                        nc.vector.tensor_scalar_add(
                            vt_sb[:, c0 : c0 + TC], pv[:], bv_t
                        )
                        for tt in range(4):
                            ti = 4 * c + tt
                            psv = ps_prj.tile(
                                [P, H], BF16, tag="pv", name=f"psv{ti}"
                            )
                            nc.tensor.transpose(
                                psv[:], vt_sb[:, ti * P : (ti + 1) * P], ident
                            )
                            nc.vector.tensor_copy(v_sb[:, ti, 0:H], psv[:])
                        vt_done = True
                    while len(emitted) - pv_i > 2 * lag:
                        drain_one(False)
                while pv_i < len(emitted):
                    j = drain_one(True)
                    if c == NTC - 1 and j == 4 * c + 1:
                        # o columns [0:256) are final once the j=4c+1 diagonal
                        # PV lands — ship them while the drain finishes
                        nc.vector.tensor_copy(
                            o_sb[:, c0 : c0 + 256], o_c[:, 0:256]
                        )
                        nc.sync.dma_start(
                            out[:, c0 : c0 + 256], o_sb[:, c0 : c0 + 256]
                        )
                if c == NTC - 1:
                    nc.vector.tensor_copy(
                        o_sb[:, c0 + 256 : c0 + TC], o_c[:, 256:TC]
                    )
                    nc.sync.dma_start(
                        out[:, c0 + 256 : c0 + TC], o_sb[:, c0 + 256 : c0 + TC]
                    )
                else:
                    nc.vector.tensor_copy(o_sb[:, c0 : c0 + TC], o_c[:])
                    nc.sync.dma_start(out[:, c0 : c0 + TC], o_sb[:, c0 : c0 + TC])
    nc.compile()
    return nc


_NC_CACHE = None


def _get_nc():
    global _NC_CACHE
    if _NC_CACHE is None:
        _NC_CACHE = build_kernel()
    return _NC_CACHE


def prep_inputs(batch_x, Wk, bk, Wq, bq, Wv, bv):
    """Host-side marshaling: per-core interleaved X^T bf16 + const blocks."""
    batch_x = np.asarray(batch_x, dtype=np.float32)
    cbh, cbf = pack_const_blocks(
        np.asarray(Wq, dtype=np.float32),
        np.asarray(Wk, dtype=np.float32),
        np.asarray(Wv, dtype=np.float32),
        np.asarray(bq, dtype=np.float32),
        np.asarray(bk, dtype=np.float32),
        np.asarray(bv, dtype=np.float32),
    )
    return [
        {"xt": pack_xt(batch_x[i]), "cbh": cbh, "cbf": cbf}
        for i in range(NCORES)
    ]


def unshard(results):
    outs = []
    for i in range(NCORES):
        o = results[i]["out"]  # [65, 2048]
        outs.append((o[:H] / o[H : H + 1]).T)  # normalize + transpose
    return np.stack(outs).astype(np.float32)


def kernel(batch_x, Wk, bk, Wq, bq, Wv, bv):
    nc = _get_nc()
    in_maps = prep_inputs(batch_x, Wk, bk, Wq, bq, Wv, bv)
    res = run_bass_kernel_spmd(nc, in_maps, list(range(NCORES)))
    return unshard(res.results)


if __name__ == "__main__":
    rng = np.random.default_rng(0)
    inputs = {
        "batch_x": rng.standard_normal((NCORES, T, E), dtype=np.float32),
        "Wk": rng.standard_normal((E, H), dtype=np.float32) * 0.03,
        "bk": rng.standard_normal((H,), dtype=np.float32) * 0.03,
        "Wq": rng.standard_normal((E, H), dtype=np.float32) * 0.03,
        "bq": rng.standard_normal((H,), dtype=np.float32) * 0.03,
        "Wv": rng.standard_normal((E, H), dtype=np.float32) * 0.03,
        "bv": rng.standard_normal((H,), dtype=np.float32) * 0.03,
    }
    out = kernel(**inputs)
    print(out.shape, out.dtype)


# revision 47
# speedup vs baseline: 1.0729x; 1.0126x over previous
"""Trainium2 Bass kernel: single-head causal attention (B=8, T=2048, E=1024, H=64).

Sharding: data-parallel over the batch dim — one batch element per NeuronCore,
8 cores, no collectives.

v10: bf16 datapath, host-side X transpose, paired ScalarE exp.
  - The host ships X^T pre-interleaved as [128, c, ec, t'] bf16 so each
    512-wide q-column is ONE dma_start with an 8KB contiguous line per
    partition. No PE transposes of X at all.
  - All matmuls run in bf16 (1 cycle/row at any moving width), accumulating
    in f32 PSUM. V^T tiles are PE-transposed back to natural layout in bf16.
  - PE clock (HAM p-state) is ramped by a warmup matmul burst on locally
    iota'd data (nonzero, varied — the activity monitor watches datapath
    toggling), no DMA dependency.
  - K^T re-base to partitions 0:63 (zero-padded to 128) is a PE selector
    matmul (E64): compute engines cannot move data across partitions, and
    an SBUF-to-SBUF DMA would queue behind the xt prefetch stream.
  - Scores are emitted in PAIRS into one 2-bank PSUM tile so a single
    ScalarE exp covers both chunks — the exp stream is the pacing engine
    in the late columns, and this halves its per-call overhead.
  - Causal masking is a bf16 0/1 multiply on the exp'd scores tile (DVE 2x).

Column sweep over 512-wide q-chunks c = 0..3:
  per column: projections QK^T/V^T (8+8 matmul chains), biases fused into
  the PSUM->SBUF copies (DVE), E64 re-base, V^T transposes; then score
  chunk-pairs (off-diagonal j first — they only need this column's Q^T —
  ending on the narrow diagonal chunks for a short drain); exp per pair on
  ScalarE straight from PSUM with the 1/sqrt(H) scale fused; PV accumulates
  into O' PSUM [65, 512] (row 64 = denominator Z) trailing the scores by
  `lag` pairs. Output per core: [65, 2048] = [unnormalized O^T; Z]; host
  divides by Z and transposes during the unshard.
"""

import numpy as np
import ml_dtypes

import concourse.bass as bass
import concourse.bacc as bacc
import concourse.mybir as mybir
from concourse.tile import TileContext
from concourse.bass_utils import run_bass_kernel_spmd

T = 2048
E = 1024
H = 64
P = 128
TC = 512  # t/q chunk width (one PSUM bank of f32)
NT = T // P  # 16 t-tiles
NE = E // P  # 8 e-chunks
NTC = T // TC  # 4 t-chunks
NCORES = 8

F32 = mybir.dt.float32
BF16 = mybir.dt.bfloat16
AF = mybir.ActivationFunctionType
BF = ml_dtypes.bfloat16

# bf16 const block column layout (per partition)
CBH_WQK = 0  # [NE * 2H] = 1024, [e_chunk, m] with m: 0:64=Wq, 64:128=Wk
CBH_WV = CBH_WQK + NE * 2 * H  # [NE * H] = 512
CBH_MASK = CBH_WV + NE * H  # [128] causal keep mask: 1.0 (y>=p) else 0.0
CBH_IDENT = CBH_MASK + P  # [64] identity for the V^T transposes
CBH_E64 = CBH_IDENT + H  # [128] selector: E64[p, m] = (p == m + 64)
CBH_BQK = CBH_E64 + P  # [2] f32 bits: bq on partitions 0:64, bk on 64:128
CBH_BV = CBH_BQK + 2  # [2] f32 bits: bv on partitions 0:64
CBH_COLS = CBH_BV + 2


def pack_const_blocks(Wq, Wk, Wv, bq, bk, bv):
    cbh = np.zeros((P, CBH_COLS), dtype=BF)
    wqk = np.zeros((P, NE, 2 * H), dtype=np.float32)
    wqk[:, :, 0:H] = Wq.reshape(NE, P, H).transpose(1, 0, 2)
    wqk[:, :, H : 2 * H] = Wk.reshape(NE, P, H).transpose(1, 0, 2)
    cbh[:, CBH_WQK : CBH_WQK + NE * 2 * H] = wqk.reshape(P, NE * 2 * H).astype(BF)
    cbh[:, CBH_WV : CBH_WV + NE * H] = (
        Wv.reshape(NE, P, H).transpose(1, 0, 2).reshape(P, NE * H).astype(BF)
    )
    p_idx = np.arange(P)[:, None]
    y_idx = np.arange(P)[None, :]
    cbh[:, CBH_MASK : CBH_MASK + P] = (y_idx >= p_idx).astype(BF)
    cbh[0:H, CBH_IDENT : CBH_IDENT + H] = np.eye(H, dtype=np.float32).astype(BF)
    cbh[:, CBH_E64 : CBH_E64 + P] = (p_idx == y_idx + H).astype(BF)
    # biases stay exact f32, stored as raw bits in two bf16 slots each
    cbh_u16 = cbh.view(np.uint16)
    bqk = np.zeros(P, dtype=np.float32)
    bqk[0:H], bqk[H : 2 * H] = bq, bk
    bqk_u = bqk.view(np.uint32)
    cbh_u16[:, CBH_BQK] = (bqk_u & 0xFFFF).astype(np.uint16)
    cbh_u16[:, CBH_BQK + 1] = (bqk_u >> 16).astype(np.uint16)
    bv_u = bv.astype(np.float32).view(np.uint32)
    cbh_u16[0:H, CBH_BV] = (bv_u & 0xFFFF).astype(np.uint16)
    cbh_u16[0:H, CBH_BV + 1] = (bv_u >> 16).astype(np.uint16)
    return cbh


def pack_xt(x):
    """[T, E] f32 -> [128, NTC*NE*TC] bf16, column-interleaved X^T so each
    512-wide column is one contiguous 8KB-per-partition DMA."""
    xt = x.T.astype(BF)  # [E, T]
    xp = xt.reshape(NE, P, NTC, TC).transpose(1, 2, 0, 3)  # [p, c, ec, t']
    return np.ascontiguousarray(xp.reshape(P, NTC * NE * TC))


def build_kernel():
    nc = bacc.Bacc("TRN2", target_bir_lowering=False, debug=False)
    xt = nc.dram_tensor("xt", [P, NTC * NE * TC], BF16, kind="ExternalInput")
    cbh = nc.dram_tensor("cbh", [P, CBH_COLS], BF16, kind="ExternalInput")
    out = nc.dram_tensor("out", [H + 1, T], F32, kind="ExternalOutput")

    with TileContext(nc) as tc:
        with (
            tc.tile_pool(name="const", bufs=1) as const,
            tc.tile_pool(name="es", bufs=4) as espool,
            tc.tile_pool(name="ps_prj", bufs=1, space="PSUM") as ps_prj,
            tc.tile_pool(name="ps_s", bufs=2, space="PSUM") as ps_s,
            tc.tile_pool(name="ps_o", bufs=2, space="PSUM") as ps_o,
        ):
            cbh_sb = const.tile([P, CBH_COLS], BF16)
            xt_all = const.tile([P, NTC, NE, TC], BF16)
            xt_t = xt.rearrange("p (c e t) -> p c e t", e=NE, t=TC)

            # warmup fodder: locally generated (no DMA dependency), nonzero
            # and varied so the PE activity monitor sees real toggling
            wtile = const.tile([P, TC], BF16)
            nc.gpsimd.iota(
                wtile[:],
                [[1, TC]],
                base=0,
                channel_multiplier=3,
                allow_small_or_imprecise_dtypes=True,
            )

            nc.sync.dma_start(cbh_sb[:], cbh[:])
            nc.sync.dma_start(xt_all[:, 0, 0:4], xt_t[:, 0, 0:4])
            nc.sync.dma_start(xt_all[:, 0, 4:NE], xt_t[:, 0, 4:NE])

            bqk_t = cbh_sb[:, CBH_BQK : CBH_BQK + 2].bitcast(F32)
            bv_t = cbh_sb[0:H, CBH_BV : CBH_BV + 2].bitcast(F32)
            wqk_sb = cbh_sb[:, CBH_WQK : CBH_WQK + NE * 2 * H].rearrange(
                "p (c m) -> p c m", m=2 * H
            )
            wv_sb = cbh_sb[:, CBH_WV : CBH_WV + NE * H].rearrange(
                "p (c m) -> p c m", m=H
            )
            maskk = cbh_sb[:, CBH_MASK : CBH_MASK + P]  # bf16 0/1 keep
            ident = cbh_sb[0:H, CBH_IDENT : CBH_IDENT + H]  # bf16 identity
            e64 = cbh_sb[:, CBH_E64 : CBH_E64 + P]  # row 64:128 selector

            # persistent activations
            qk_sb = const.tile([P, T], BF16)  # rows 0:64 = Q^T, 64:128 = K^T
            kt_sb = const.tile([P, T], BF16)  # K^T re-based, rows 64:128 zero
            vt_sb = const.tile([H, T], BF16)  # V^T staging
            v_sb = const.tile([P, NT, H + 1], BF16)  # V' = [V, 1] natural
            o_sb = const.tile([H + 1, T], F32)
            # ones column of V'
            nc.vector.memset(v_sb[:, :, H], 1.0)

            # HAM warmup: ramp the PE clock while DMAs stream in
            warm = ps_s.tile([P, 2 * TC], F32, tag="s")
            for _ in range(15):
                nc.tensor.matmul(
                    warm[:, 0:TC], wtile[0:P, 0:P], wtile[:],
                    start=True, stop=True,
                )

            scale = 1.0 / np.sqrt(np.float32(H))
            es_tiles = {}

            def chunk_geom(j, c):
                k0 = j * P
                q0 = max(c * TC, k0)
                return k0, q0, (c + 1) * TC - q0

            def emit_score_pair(ja, jb, c):
                """Two score chunks into one 2-bank PSUM tile, one exp."""
                ps2 = ps_s.tile([P, 2 * TC], F32, tag="s", name=f"s{ja}_{c}")
                es2 = espool.tile([P, 2 * TC], BF16, tag="es", name=f"e{ja}_{c}")
                base = 0
                for j in (ja, jb):
                    if j is None:
                        continue
                    k0, q0, w = chunk_geom(j, c)
                    es_tiles[(j, c)] = (es2, base, q0, w)
                    nc.tensor.matmul(
                        ps2[:, base : base + w],
                        kt_sb[:, k0 : k0 + P],
                        qk_sb[:, q0 : q0 + w],
                        start=True,
                        stop=True,
                    )
                    base += w
                nc.scalar.activation(
                    es2[:, 0:base], ps2[:, 0:base], AF.Exp, scale=float(scale)
                )
                for j in (ja, jb):
                    if j is None:
                        continue
                    es2, b, q0, w = es_tiles[(j, c)]
                    if q0 == j * P:
                        # causal mask inside the diagonal 128x128 block
                        nc.vector.tensor_tensor(
                            es2[:, b : b + P],
                            es2[:, b : b + P],
                            maskk,
                            mybir.AluOpType.mult,
                        )

            def emit_pv(jc, c, o_c, first, last):
                j, _ = jc
                es2, b, q0, w = es_tiles.pop(jc)
                nc.tensor.matmul(
                    o_c[:, q0 - c * TC : q0 - c * TC + w],
                    v_sb[:, j, :],
                    es2[:, b : b + w],
                    start=first,
                    stop=last,
                )

            for c in range(NTC):
                c0 = c * TC
                # stream in the NEXT column's x
                if c < NTC - 1:
                    nc.sync.dma_start(xt_all[:, c + 1], xt_t[:, c + 1])
                # QK projection (the V chain is emitted after the first score
                # pair so the exp stream — the pacing engine — starts sooner)
                pqk = ps_prj.tile([P, TC], F32, tag="pqk", name=f"pqk{c}")
                for ec in range(NE):
                    nc.tensor.matmul(
                        pqk[:],
                        wqk_sb[:, ec, :],
                        xt_all[:, c, ec, :],
                        start=(ec == 0),
                        stop=(ec == NE - 1),
                    )
                nc.vector.tensor_scalar_add(qk_sb[:, c0 : c0 + TC], pqk[:], bqk_t)
                # K^T re-base via a PE selector matmul (E64 picks rows 64:128
                # and zero-pads the rest); reuses the pqk PSUM bank
                pkt = ps_prj.tile([P, TC], F32, tag="pqk", name=f"pkt{c}")
                nc.tensor.matmul(
                    pkt[:], e64, qk_sb[:, c0 : c0 + TC], start=True, stop=True
                )
                nc.vector.tensor_copy(kt_sb[:, c0 : c0 + TC], pkt[:])

                # chunk pairs: off-diagonal first (they only need this
                # column's Q^T), ending on the narrow diagonal chunks
                order = list(range(4 * c + 4))
                pairs = [
                    (order[i], order[i + 1] if i + 1 < len(order) else None)
                    for i in range(0, len(order), 2)
                ]
                o_c = ps_o.tile([H + 1, TC], F32, tag="o", name=f"o{c}")
                lag = 2  # PV trails scores by `lag` pairs
                vt_done = False
                emitted = []
                pv_i = 0

                def drain_one(last_allowed):
                    nonlocal pv_i
                    j = emitted[pv_i][0]
                    emit_pv(
                        emitted[pv_i], c, o_c, pv_i == 0,
                        last_allowed and pv_i == len(emitted) - 1,
                    )
                    pv_i += 1
                    return j

                for pi, (ja, jb) in enumerate(pairs):
                    emit_score_pair(ja, jb, c)
                    emitted.append((ja, c))
                    if jb is not None:
                        emitted.append((jb, c))
                    if not vt_done:
                        # V projection + V' transposes tucked behind the
                        # first score pair
                        pv = ps_prj.tile([H, TC], F32, tag="pv", name=f"pv{c}")
                        for ec in range(NE):
                            nc.tensor.matmul(
                                pv[:],
                                wv_sb[:, ec, :],
                                xt_all[:, c, ec, :],
                                start=(ec == 0),
                                stop=(ec == NE - 1),
                            )
                        nc.vector.tensor_scalar_add(
                            vt_sb[:, c0 : c0 + TC], pv[:], bv_t
                        )
                        for tt in range(4):
                            ti = 4 * c + tt
                            psv = ps_prj.tile(
                                [P, H], BF16, tag="pv", name=f"psv{ti}"
                            )
                            nc.tensor.transpose(
                                psv[:], vt_sb[:, ti * P : (ti + 1) * P], ident
                            )
                            nc.vector.tensor_copy(v_sb[:, ti, 0:H], psv[:])
                        vt_done = True
                    while len(emitted) - pv_i > 2 * lag:
                        drain_one(False)
                while pv_i < len(emitted):
                    j = drain_one(True)
                    if c == NTC - 1 and j == 4 * c + 1:
                        # o columns [0:256) are final once the j=4c+1 diagonal
                        # PV lands — ship them while the drain finishes
                        nc.vector.tensor_copy(
                            o_sb[:, c0 : c0 + 256], o_c[:, 0:256]
                        )
                        nc.sync.dma_start(
                            out[:, c0 : c0 + 256], o_sb[:, c0 : c0 + 256]
                        )
                if c == NTC - 1:
                    nc.vector.tensor_copy(
                        o_sb[:, c0 + 256 : c0 + TC], o_c[:, 256:TC]
                    )
                    nc.sync.dma_start(
                        out[:, c0 + 256 : c0 + TC], o_sb[:, c0 + 256 : c0 + TC]
                    )
                else:
                    nc.vector.tensor_copy(o_sb[:, c0 : c0 + TC], o_c[:])
                    nc.sync.dma_start(out[:, c0 : c0 + TC], o_sb[:, c0 : c0 + TC])
    nc.compile()
    return nc


_NC_CACHE = None


def _get_nc():
    global _NC_CACHE
    if _NC_CACHE is None:
        _NC_CACHE = build_kernel()
    return _NC_CACHE


def prep_inputs(batch_x, Wk, bk, Wq, bq, Wv, bv):
    """Host-side marshaling: per-core interleaved X^T bf16 + const blocks."""
    batch_x = np.asarray(batch_x, dtype=np.float32)
    cbh = pack_const_blocks(
        np.asarray(Wq, dtype=np.float32),
        np.asarray(Wk, dtype=np.float32),
        np.asarray(Wv, dtype=np.float32),
        np.asarray(bq, dtype=np.float32),
        np.asarray(bk, dtype=np.float32),
        np.asarray(bv, dtype=np.float32),
    )
    return [
        {"xt": pack_xt(batch_x[i]), "cbh": cbh} for i in range(NCORES)
    ]


def unshard(results):
    outs = []
    for i in range(NCORES):
        o = results[i]["out"]  # [65, 2048]
        outs.append((o[:H] / o[H : H + 1]).T)  # normalize + transpose
    return np.stack(outs).astype(np.float32)


def kernel(batch_x, Wk, bk, Wq, bq, Wv, bv):
    nc = _get_nc()
    in_maps = prep_inputs(batch_x, Wk, bk, Wq, bq, Wv, bv)
    res = run_bass_kernel_spmd(nc, in_maps, list(range(NCORES)))
    return unshard(res.results)


if __name__ == "__main__":
    rng = np.random.default_rng(0)
    inputs = {
        "batch_x": rng.standard_normal((NCORES, T, E), dtype=np.float32),
        "Wk": rng.standard_normal((E, H), dtype=np.float32) * 0.03,
        "bk": rng.standard_normal((H,), dtype=np.float32) * 0.03,
        "Wq": rng.standard_normal((E, H), dtype=np.float32) * 0.03,
        "bq": rng.standard_normal((H,), dtype=np.float32) * 0.03,
        "Wv": rng.standard_normal((E, H), dtype=np.float32) * 0.03,
        "bv": rng.standard_normal((H,), dtype=np.float32) * 0.03,
    }
    out = kernel(**inputs)
    print(out.shape, out.dtype)
